# revision 7
# baseline (speedup 1.0000x reference)
"""QMIX-style MixingNetwork Trainium2 kernel.

Shapes (hardcoded from the problem spec):
  B, S, A, C, E, H = 256, 512, 8, 256, 64, 256
  agent_q_values [B, S, A], central_states [B, S, C] -> out [B, S, 1]

Strategy: pure data parallel over the batch dim across 8 NeuronCores
(n_c = B*S/8 = 16384 samples per core). All compute runs in a
"transposed" activation layout (features on SBUF partitions, samples on
the free dim) so that hypernet biases are per-partition scalars and all
GEMMs keep the contraction dim on partitions.

Per core, per 512-sample chunk (32 chunks):
  GEMM1: z.T[832, n] = [w1a|w2a|wb2a|wb1].T @ st.T           (14 f32r MMs)
  ELU:   hp = elu(z+b)+1 = max(z+b+1, exp(min(z+b, 0)))       (ACT exp + DVE)
  GEMM2: w1raw.T[512,n], w2raw.T[64,n], b2.T[1,n] from hp     (12 MMs)
  |.|+bias (DVE fused tensor_scalar add+abs_max)
  prod = |w1| * qb   (qb = q broadcast, precomputed host-side, via DMA)
  a-sum: PE matmul against 0/1 selection matrix -> hidden_pre[64, n]
  b1 add via identity matmul, relu, dot with w2 via DVE + PE ones-matmul
  out[1, n] = psum + const

The "+1" offset trick: hp = elu+1 avoids a separate "-1" pass; the
constant offset is folded into the next layer's bias host-side
(b' = b - colsum(W)).
"""

import os
import sys

for _p in ("/opt/trn_rl_repo", "/root/.axon_site/_ro/trn_rl_repo"):
    if os.path.isdir(_p) and _p not in sys.path:
        sys.path.append(_p)

from contextlib import ExitStack

import numpy as np

import concourse.bass as bass
import concourse.mybir as mybir
import concourse.tile as tile
from concourse import bacc
from concourse.bass_utils import run_bass_kernel_spmd

B, S, A, C, E, H = 256, 512, 8, 256, 64, 256
N_CORES = 8
NC_SAMPLES = B * S // N_CORES        # 16384 samples per core
CHUNK = 512                          # samples per inner chunk
N_CHUNKS = NC_SAMPLES // CHUNK       # 32
F = 3 * H + E                        # 832 fused GEMM1 output dim
AE = A * E                           # 512

FP32 = mybir.dt.float32
BF16 = mybir.dt.bfloat16
F32R = mybir.dt.float32r
ALU = mybir.AluOpType
AF = mybir.ActivationFunctionType

# bias_pack row layout (each row is one 128-wide per-partition scalar col)
ROW_NEGB = 0      # 6 rows: -(b) for ELU tiles f=0..5
ROW_B = 6         # 6 rows: b      (exp bias)
ROW_BP1 = 12      # 6 rows: b + 1  (max operand bias)
ROW_B1B = 18      # 4 rows: b1b - colsum(w1b), tiles m=0..3
ROW_B2B = 22      # 1 row:  b2b - colsum(w2b) in first 64 entries
ROW_BB1 = 23      # 1 row:  bb1 in first 64 entries
N_BIAS_ROWS = 24


def _build_nc():
    nc = bacc.Bacc("TRN2", target_bir_lowering=False, debug=False)

    xt = nc.dram_tensor("xt", [C, NC_SAMPLES], F32R, kind="ExternalInput")
    qb = nc.dram_tensor("qb", [AE, NC_SAMPLES], BF16, kind="ExternalInput")
    wcat = nc.dram_tensor("wcat", [C, F], F32R, kind="ExternalInput")
    w1b = nc.dram_tensor("w1b", [H, AE], BF16, kind="ExternalInput")
    w2b = nc.dram_tensor("w2b", [H, E], BF16, kind="ExternalInput")
    wb2b = nc.dram_tensor("wb2b", [H, 1], BF16, kind="ExternalInput")
    biases = nc.dram_tensor("biases", [N_BIAS_ROWS, 128], FP32, kind="ExternalInput")
    konst = nc.dram_tensor("konst", [128, 129], BF16, kind="ExternalInput")
    out = nc.dram_tensor("out", [1, NC_SAMPLES], FP32, kind="ExternalOutput")
    out_bias = nc.dram_tensor("out_bias", [1, 1], FP32, kind="ExternalInput")

    with ExitStack() as ctx:
        tc = ctx.enter_context(tile.TileContext(nc))
        singles = ctx.enter_context(tc.tile_pool(name="singles", bufs=1))
        xpool = ctx.enter_context(tc.tile_pool(name="xpool", bufs=3))
        qpool = ctx.enter_context(tc.tile_pool(name="qpool", bufs=2))
        spool = ctx.enter_context(tc.tile_pool(name="spool", bufs=3))   # a, e
        hpool = ctx.enter_context(tc.tile_pool(name="hpool", bufs=2))   # hp (6 tags)
        mpool = ctx.enter_context(tc.tile_pool(name="mpool", bufs=3))   # absw1, prod
        tpool = ctx.enter_context(tc.tile_pool(name="tpool", bufs=2))   # small [64,n]
        zps = ctx.enter_context(tc.tile_pool(name="zps", bufs=2, space="PSUM"))
        w1ps = ctx.enter_context(tc.tile_pool(name="w1ps", bufs=2, space="PSUM"))
        w2ps = ctx.enter_context(tc.tile_pool(name="w2ps", bufs=1, space="PSUM"))
        jps = ctx.enter_context(tc.tile_pool(name="jps", bufs=2, space="PSUM"))
        hps = ctx.enter_context(tc.tile_pool(name="hps", bufs=1, space="PSUM"))

        # ---- preload weights/constants ----
        wcat_sb = []
        w1b_sb = []
        w2b_sb = []
        wb2b_sb = []
        for k in range(2):
            t = singles.tile([128, F], F32R, tag=f"wcat{k}")
            nc.sync.dma_start(out=t, in_=wcat[k * 128:(k + 1) * 128, :])
            wcat_sb.append(t)
            t = singles.tile([128, AE], BF16, tag=f"w1b{k}")
            nc.sync.dma_start(out=t, in_=w1b[k * 128:(k + 1) * 128, :])
            w1b_sb.append(t)
            t = singles.tile([128, E], BF16, tag=f"w2b{k}")
            nc.sync.dma_start(out=t, in_=w2b[k * 128:(k + 1) * 128, :])
            w2b_sb.append(t)
            t = singles.tile([128, 1], BF16, tag=f"wb2b{k}")
            nc.sync.dma_start(out=t, in_=wb2b[k * 128:(k + 1) * 128, :])
            wb2b_sb.append(t)
        bias_sb = singles.tile([128, N_BIAS_ROWS], FP32, tag="bias")
        nc.sync.dma_start(
            out=bias_sb,
            in_=bass.AP(tensor=biases, offset=0,
                        ap=[[1, 128], [128, N_BIAS_ROWS]]),
        )
        konst_sb = singles.tile([128, 129], BF16, tag="konst")
        nc.sync.dma_start(out=konst_sb, in_=konst[:, :])
        ob_sb = singles.tile([1, 1], FP32, tag="ob")
        nc.sync.dma_start(out=ob_sb, in_=out_bias[:, :])
        out_sb = singles.tile([1, NC_SAMPLES], FP32, tag="out")

        sel = konst_sb[:, 0:64]        # Sel[p, e] = (p % 64 == e)
        i64 = konst_sb[0:64, 64:128]   # identity 64
        ones64 = konst_sb[0:64, 128:129]

        def bcol(row, parts=128):
            return bias_sb[0:parts, row:row + 1]

        for ci in range(N_CHUNKS):
            cs = slice(ci * CHUNK, (ci + 1) * CHUNK)

            xts = []
            for k in range(2):
                t = xpool.tile([128, CHUNK], F32R, tag=f"xt{k}")
                nc.sync.dma_start(out=t, in_=xt[k * 128:(k + 1) * 128, cs])
                xts.append(t)
            qbs = []
            for m in range(4):
                t = qpool.tile([128, CHUNK], BF16, tag=f"qb{m}")
                nc.sync.dma_start(out=t, in_=qb[m * 128:(m + 1) * 128, cs])
                qbs.append(t)

            # ---- GEMM1 + ELU ----
            hp = []
            for f in range(7):
                fw = 128 if f < 6 else 64
                z = zps.tile([fw, CHUNK], FP32, tag="z")
                for k in range(2):
                    nc.tensor.matmul(
                        z, wcat_sb[k][:, f * 128:f * 128 + fw], xts[k],
                        start=(k == 0), stop=(k == 1),
                    )
                if f < 6:
                    # hp = elu(z+b)+1 = max(z+(b+1), min(exp(z+b), 1))
                    e = spool.tile([128, CHUNK], BF16, tag="e")
                    nc.scalar.activation(e, z, AF.Exp, bias=bcol(ROW_B + f))
                    em = spool.tile([128, CHUNK], BF16, tag="em")
                    nc.gpsimd.tensor_scalar(em, e, 1.0, None, ALU.min)
                    t = hpool.tile([128, CHUNK], BF16, tag=f"hp{f}")
                    nc.vector.scalar_tensor_tensor(
                        t, z, bcol(ROW_BP1 + f), em, op0=ALU.add, op1=ALU.max)
                    hp.append(t)
                else:
                    # b1 branch: b1t = z + bb1
                    b1t = tpool.tile([64, CHUNK], BF16, tag="b1t")
                    nc.scalar.activation(b1t, z, AF.Identity,
                                         bias=bias_sb[0:64, ROW_BB1:ROW_BB1 + 1])

            # ---- GEMM2 + abs + prod + a-sum ----
            psh = hps.tile([64, CHUNK], FP32, tag="psh")
            for m in range(4):
                pw = w1ps.tile([128, CHUNK], FP32, tag="pw")
                for k in range(2):
                    nc.tensor.matmul(
                        pw, w1b_sb[k][:, m * 128:(m + 1) * 128],
                        hp[k],
                        start=(k == 0), stop=(k == 1),
                    )
                aw = mpool.tile([128, CHUNK], BF16, tag="aw")
                nc.scalar.activation(aw, pw, AF.Abs, bias=bcol(ROW_B1B + m))
                pr = mpool.tile([128, CHUNK], BF16, tag="pr")
                nc.vector.tensor_tensor(pr, aw, qbs[m], ALU.mult)
                nc.tensor.matmul(psh, sel, pr,
                                 start=(m == 0), stop=False,
                                 skip_group_check=True)

            pj = jps.tile([1, CHUNK], FP32, tag="pj")
            pw2 = w2ps.tile([64, CHUNK], FP32, tag="pw2")
            for k in range(2):
                nc.tensor.matmul(pw2, w2b_sb[k], hp[2 + k],
                                 start=(k == 0), stop=(k == 1))
                nc.tensor.matmul(pj, wb2b_sb[k], hp[4 + k],
                                 start=(k == 0), stop=False,
                                 skip_group_check=True)
            w2t = tpool.tile([64, CHUNK], BF16, tag="w2t")
            nc.scalar.activation(w2t, pw2, AF.Abs,
                                 bias=bias_sb[0:64, ROW_B2B:ROW_B2B + 1])

            # b1 add (identity matmul) then relu -> hidden
            nc.tensor.matmul(psh, i64, b1t,
                             start=False, stop=True, skip_group_check=True)
            hid = tpool.tile([64, CHUNK], BF16, tag="hid")
            nc.vector.tensor_scalar(hid, psh, 0.0, None, ALU.max)
            dotp = tpool.tile([64, CHUNK], BF16, tag="dotp")
            nc.vector.tensor_tensor(dotp, hid, w2t, ALU.mult)
            nc.tensor.matmul(pj, ones64, dotp,
                             start=False, stop=True, skip_group_check=True)

            # out = pj + (bb2b - colsum(wb2b))
            nc.vector.tensor_scalar(out_sb[0:1, cs], pj, ob_sb[0:1, 0:1],
                                    None, ALU.add)

        nc.sync.dma_start(out=out[:, :], in_=out_sb)

    nc.compile()
    return nc


_NC_CACHE = None


def _get_nc():
    global _NC_CACHE
    if _NC_CACHE is None:
        _NC_CACHE = _build_nc()
    return _NC_CACHE


def _prep_core_inputs(agent_q_values, central_states, weights):
    """Build per-core input maps (layout transforms only)."""
    st = central_states.reshape(B * S, C)
    q = agent_q_values.reshape(B * S, A)

    (w1a, b1a, w1b, b1b, w2a, b2a, w2b, b2b,
     wb1, bb1, wb2a, bb2a, wb2b, bb2b) = weights

    wcat = np.concatenate([w1a, w2a, wb2a, wb1], axis=1)          # [C, 832]
    bcat = np.concatenate([b1a, b2a, bb2a])                        # [768]

    bias_pack = np.zeros((N_BIAS_ROWS, 128), np.float32)
    for f in range(6):
        seg = bcat[f * 128:(f + 1) * 128]
        bias_pack[ROW_NEGB + f] = -seg
        bias_pack[ROW_B + f] = seg
        bias_pack[ROW_BP1 + f] = seg + 1.0
    b1bp = b1b - w1b.sum(axis=0)
    for m in range(4):
        bias_pack[ROW_B1B + m] = b1bp[m * 128:(m + 1) * 128]
    bias_pack[ROW_B2B, 0:64] = b2b - w2b.sum(axis=0)
    bias_pack[ROW_BB1, 0:64] = bb1

    out_bias = np.array([[bb2b[0] - wb2b.sum()]], np.float32)

    konst = np.zeros((128, 129), np.float32)
    p = np.arange(128)
    konst[p, p % 64] = 1.0                    # Sel
    konst[np.arange(64), 64 + np.arange(64)] = 1.0   # I64
    konst[0:64, 128] = 1.0                    # ones

    import ml_dtypes
    bf16 = np.dtype(ml_dtypes.bfloat16)
    shared = dict(
        wcat=np.ascontiguousarray(wcat, np.float32),
        w1b=np.ascontiguousarray(w1b).astype(bf16),
        w2b=np.ascontiguousarray(w2b).astype(bf16),
        wb2b=np.ascontiguousarray(wb2b).astype(bf16),
        biases=bias_pack, konst=konst.astype(bf16), out_bias=out_bias,
    )

    in_maps = []
    for c in range(N_CORES):
        sl = slice(c * NC_SAMPLES, (c + 1) * NC_SAMPLES)
        xt_c = np.ascontiguousarray(st[sl].T, np.float32)          # [C, n]
        qT = np.ascontiguousarray(q[sl].T, np.float32)             # [A, n]
        qb_c = np.repeat(qT, E, axis=0).astype(bf16)               # [AE, n]
        in_maps.append(dict(xt=xt_c, qb=np.ascontiguousarray(qb_c), **shared))
    return in_maps


def kernel(agent_q_values, central_states,
           w1a, b1a, w1b, b1b, w2a, b2a, w2b, b2b,
           wb1, bb1, wb2a, bb2a, wb2b, bb2b, _trace=False, _result_box=None):
    nc = _get_nc()
    weights = (w1a, b1a, w1b, b1b, w2a, b2a, w2b, b2b,
               wb1, bb1, wb2a, bb2a, wb2b, bb2b)
    weights = tuple(np.asarray(w, np.float32) for w in weights)
    in_maps = _prep_core_inputs(
        np.asarray(agent_q_values, np.float32),
        np.asarray(central_states, np.float32), weights)

    res = run_bass_kernel_spmd(nc, in_maps, core_ids=list(range(N_CORES)),
                               trace=_trace)
    if _result_box is not None:
        _result_box.append(res)

    out = np.concatenate(
        [res.results[c]["out"].reshape(NC_SAMPLES) for c in range(N_CORES)])
    return out.reshape(B, S, 1).astype(np.float32)


# revision 8
# speedup vs baseline: 3.7888x; 3.7888x over previous
"""QMIX-style MixingNetwork Trainium2 kernel.

Shapes (hardcoded from the problem spec):
  B, S, A, C, E, H = 256, 512, 8, 256, 64, 256
  agent_q_values [B, S, A], central_states [B, S, C] -> out [B, S, 1]

Strategy: pure data parallel over the batch dim across 8 NeuronCores
(n_c = B*S/8 = 16384 samples per core). All compute runs in a
"transposed" activation layout (features on SBUF partitions, samples on
the free dim) so that hypernet biases are per-partition scalars and all
GEMMs keep the contraction dim on partitions.

Per core, per 512-sample chunk (32 chunks):
  GEMM1: z.T[832, n] = [w1a|w2a|wb2a|wb1].T @ st.T           (14 f32r MMs)
  ELU:   hp = elu(z+b)+1 = max(z+b+1, exp(min(z+b, 0)))       (ACT exp + DVE)
  GEMM2: w1raw.T[512,n], w2raw.T[64,n], b2.T[1,n] from hp     (12 MMs)
  |.|+bias (DVE fused tensor_scalar add+abs_max)
  prod = |w1| * qb   (qb = q broadcast, precomputed host-side, via DMA)
  a-sum: PE matmul against 0/1 selection matrix -> hidden_pre[64, n]
  b1 add via identity matmul, relu, dot with w2 via DVE + PE ones-matmul
  out[1, n] = psum + const

The "+1" offset trick: hp = elu+1 avoids a separate "-1" pass; the
constant offset is folded into the next layer's bias host-side
(b' = b - colsum(W)).
"""

import os
import sys

for _p in ("/opt/trn_rl_repo", "/root/.axon_site/_ro/trn_rl_repo"):
    if os.path.isdir(_p) and _p not in sys.path:
        sys.path.append(_p)

from contextlib import ExitStack

import numpy as np

import concourse.bass as bass
import concourse.mybir as mybir
import concourse.tile as tile
from concourse import bacc
from concourse.bass_utils import run_bass_kernel_spmd

B, S, A, C, E, H = 256, 512, 8, 256, 64, 256
N_CORES = 8
NC_SAMPLES = B * S // N_CORES        # 16384 samples per core
CHUNK = 512                          # samples per inner chunk
N_CHUNKS = NC_SAMPLES // CHUNK       # 32
F = 3 * H + E                        # 832 fused GEMM1 output dim
AE = A * E                           # 512

FP32 = mybir.dt.float32
BF16 = mybir.dt.bfloat16
F32R = mybir.dt.float32r
ALU = mybir.AluOpType
AF = mybir.ActivationFunctionType

# bias_pack row layout (each row is one 128-wide per-partition scalar col)
ROW_NEGB = 0      # 6 rows: -(b) for ELU tiles f=0..5
ROW_B = 6         # 6 rows: b      (exp bias)
ROW_BP1 = 12      # 6 rows: b + 1  (max operand bias)
ROW_B1B = 18      # 4 rows: b1b - colsum(w1b), tiles m=0..3
ROW_B2B = 22      # 1 row:  b2b - colsum(w2b) in first 64 entries
ROW_BB1 = 23      # 1 row:  bb1 in first 64 entries
N_BIAS_ROWS = 24


def _build_nc():
    nc = bacc.Bacc("TRN2", target_bir_lowering=False, debug=False)

    xt = nc.dram_tensor("xt", [C, NC_SAMPLES], F32R, kind="ExternalInput")
    qb = nc.dram_tensor("qb", [AE, NC_SAMPLES], BF16, kind="ExternalInput")
    wcat = nc.dram_tensor("wcat", [C, F], F32R, kind="ExternalInput")
    w1b = nc.dram_tensor("w1b", [H, AE], BF16, kind="ExternalInput")
    w2b = nc.dram_tensor("w2b", [H, E], BF16, kind="ExternalInput")
    wb2b = nc.dram_tensor("wb2b", [H, 1], BF16, kind="ExternalInput")
    biases = nc.dram_tensor("biases", [N_BIAS_ROWS, 128], FP32, kind="ExternalInput")
    konst = nc.dram_tensor("konst", [128, 129], BF16, kind="ExternalInput")
    out = nc.dram_tensor("out", [1, NC_SAMPLES], FP32, kind="ExternalOutput")
    out_bias = nc.dram_tensor("out_bias", [1, 1], FP32, kind="ExternalInput")

    with ExitStack() as ctx:
        tc = ctx.enter_context(tile.TileContext(nc))
        singles = ctx.enter_context(tc.tile_pool(name="singles", bufs=1))
        xpool = ctx.enter_context(tc.tile_pool(name="xpool", bufs=3))
        qpool = ctx.enter_context(tc.tile_pool(name="qpool", bufs=2))
        spool = ctx.enter_context(tc.tile_pool(name="spool", bufs=3))   # a, e
        hpool = ctx.enter_context(tc.tile_pool(name="hpool", bufs=2))   # hp (6 tags)
        mpool = ctx.enter_context(tc.tile_pool(name="mpool", bufs=3))   # absw1, prod
        tpool = ctx.enter_context(tc.tile_pool(name="tpool", bufs=2))   # small [64,n]
        zps = ctx.enter_context(tc.tile_pool(name="zps", bufs=2, space="PSUM"))
        w1ps = ctx.enter_context(tc.tile_pool(name="w1ps", bufs=2, space="PSUM"))
        w2ps = ctx.enter_context(tc.tile_pool(name="w2ps", bufs=1, space="PSUM"))
        jps = ctx.enter_context(tc.tile_pool(name="jps", bufs=2, space="PSUM"))
        hps = ctx.enter_context(tc.tile_pool(name="hps", bufs=1, space="PSUM"))

        # ---- preload weights/constants ----
        wcat_sb = []
        w1b_sb = []
        w2b_sb = []
        wb2b_sb = []
        for k in range(2):
            t = singles.tile([128, F], F32R, tag=f"wcat{k}")
            nc.sync.dma_start(out=t, in_=wcat[k * 128:(k + 1) * 128, :])
            wcat_sb.append(t)
            t = singles.tile([128, AE], BF16, tag=f"w1b{k}")
            nc.sync.dma_start(out=t, in_=w1b[k * 128:(k + 1) * 128, :])
            w1b_sb.append(t)
            t = singles.tile([128, E], BF16, tag=f"w2b{k}")
            nc.sync.dma_start(out=t, in_=w2b[k * 128:(k + 1) * 128, :])
            w2b_sb.append(t)
            t = singles.tile([128, 1], BF16, tag=f"wb2b{k}")
            nc.sync.dma_start(out=t, in_=wb2b[k * 128:(k + 1) * 128, :])
            wb2b_sb.append(t)
        bias_sb = singles.tile([128, N_BIAS_ROWS], FP32, tag="bias")
        nc.sync.dma_start(
            out=bias_sb,
            in_=bass.AP(tensor=biases, offset=0,
                        ap=[[1, 128], [128, N_BIAS_ROWS]]),
        )
        konst_sb = singles.tile([128, 129], BF16, tag="konst")
        nc.sync.dma_start(out=konst_sb, in_=konst[:, :])
        ob_sb = singles.tile([1, 1], FP32, tag="ob")
        nc.sync.dma_start(out=ob_sb, in_=out_bias[:, :])
        out_sb = singles.tile([1, NC_SAMPLES], FP32, tag="out")

        sel = konst_sb[:, 0:64]        # Sel[p, e] = (p % 64 == e)
        i64 = konst_sb[0:64, 64:128]   # identity 64
        ones64 = konst_sb[0:64, 128:129]

        def bcol(row, parts=128):
            return bias_sb[0:parts, row:row + 1]

        for ci in range(N_CHUNKS):
            cs = slice(ci * CHUNK, (ci + 1) * CHUNK)

            xts = []
            for k in range(2):
                t = xpool.tile([128, CHUNK], F32R, tag=f"xt{k}")
                nc.sync.dma_start(out=t, in_=xt[k * 128:(k + 1) * 128, cs])
                xts.append(t)
            qbs = []
            for m in range(4):
                t = qpool.tile([128, CHUNK], BF16, tag=f"qb{m}")
                nc.sync.dma_start(out=t, in_=qb[m * 128:(m + 1) * 128, cs])
                qbs.append(t)

            # ---- GEMM1 + ELU ----
            hp = []
            for f in range(7):
                fw = 128 if f < 6 else 64
                z = zps.tile([fw, CHUNK], FP32, tag="z")
                for k in range(2):
                    nc.tensor.matmul(
                        z, wcat_sb[k][:, f * 128:f * 128 + fw], xts[k],
                        start=(k == 0), stop=(k == 1),
                    )
                if f < 6:
                    # hp = elu(z+b)+1 = max(z+(b+1), min(exp(z+b), 1))
                    e = spool.tile([128, CHUNK], BF16, tag="e")
                    nc.scalar.activation(e, z, AF.Exp, bias=bcol(ROW_B + f))
                    em = spool.tile([128, CHUNK], BF16, tag="em")
                    nc.vector.tensor_scalar(em, e, 1.0, None, ALU.min)
                    t = hpool.tile([128, CHUNK], BF16, tag=f"hp{f}")
                    nc.vector.scalar_tensor_tensor(
                        t, z, bcol(ROW_BP1 + f), em, op0=ALU.add, op1=ALU.max)
                    hp.append(t)
                else:
                    # b1 branch: b1t = z + bb1
                    b1t = tpool.tile([64, CHUNK], BF16, tag="b1t")
                    nc.scalar.activation(b1t, z, AF.Identity,
                                         bias=bias_sb[0:64, ROW_BB1:ROW_BB1 + 1])

            # ---- GEMM2 + abs + prod + a-sum ----
            psh = hps.tile([64, CHUNK], FP32, tag="psh")
            for m in range(4):
                pw = w1ps.tile([128, CHUNK], FP32, tag="pw")
                for k in range(2):
                    nc.tensor.matmul(
                        pw, w1b_sb[k][:, m * 128:(m + 1) * 128],
                        hp[k],
                        start=(k == 0), stop=(k == 1),
                    )
                aw = mpool.tile([128, CHUNK], BF16, tag="aw")
                nc.scalar.activation(aw, pw, AF.Abs, bias=bcol(ROW_B1B + m))
                pr = mpool.tile([128, CHUNK], BF16, tag="pr")
                nc.vector.tensor_tensor(pr, aw, qbs[m], ALU.mult)
                nc.tensor.matmul(psh, sel, pr,
                                 start=(m == 0), stop=False,
                                 skip_group_check=True)

            pj = jps.tile([1, CHUNK], FP32, tag="pj")
            pw2 = w2ps.tile([64, CHUNK], FP32, tag="pw2")
            for k in range(2):
                nc.tensor.matmul(pw2, w2b_sb[k], hp[2 + k],
                                 start=(k == 0), stop=(k == 1))
                nc.tensor.matmul(pj, wb2b_sb[k], hp[4 + k],
                                 start=(k == 0), stop=False,
                                 skip_group_check=True)
            w2t = tpool.tile([64, CHUNK], BF16, tag="w2t")
            nc.scalar.activation(w2t, pw2, AF.Abs,
                                 bias=bias_sb[0:64, ROW_B2B:ROW_B2B + 1])

            # b1 add (identity matmul) then relu -> hidden
            nc.tensor.matmul(psh, i64, b1t,
                             start=False, stop=True, skip_group_check=True)
            hid = tpool.tile([64, CHUNK], BF16, tag="hid")
            nc.vector.tensor_scalar(hid, psh, 0.0, None, ALU.max)
            dotp = tpool.tile([64, CHUNK], BF16, tag="dotp")
            nc.vector.tensor_tensor(dotp, hid, w2t, ALU.mult)
            nc.tensor.matmul(pj, ones64, dotp,
                             start=False, stop=True, skip_group_check=True)

            # out = pj + (bb2b - colsum(wb2b))
            nc.vector.tensor_scalar(out_sb[0:1, cs], pj, ob_sb[0:1, 0:1],
                                    None, ALU.add)

        nc.sync.dma_start(out=out[:, :], in_=out_sb)

    nc.compile()
    return nc


_NC_CACHE = None


def _get_nc():
    global _NC_CACHE
    if _NC_CACHE is None:
        _NC_CACHE = _build_nc()
    return _NC_CACHE


def _prep_core_inputs(agent_q_values, central_states, weights):
    """Build per-core input maps (layout transforms only)."""
    st = central_states.reshape(B * S, C)
    q = agent_q_values.reshape(B * S, A)

    (w1a, b1a, w1b, b1b, w2a, b2a, w2b, b2b,
     wb1, bb1, wb2a, bb2a, wb2b, bb2b) = weights

    wcat = np.concatenate([w1a, w2a, wb2a, wb1], axis=1)          # [C, 832]
    bcat = np.concatenate([b1a, b2a, bb2a])                        # [768]

    bias_pack = np.zeros((N_BIAS_ROWS, 128), np.float32)
    for f in range(6):
        seg = bcat[f * 128:(f + 1) * 128]
        bias_pack[ROW_NEGB + f] = -seg
        bias_pack[ROW_B + f] = seg
        bias_pack[ROW_BP1 + f] = seg + 1.0
    b1bp = b1b - w1b.sum(axis=0)
    for m in range(4):
        bias_pack[ROW_B1B + m] = b1bp[m * 128:(m + 1) * 128]
    bias_pack[ROW_B2B, 0:64] = b2b - w2b.sum(axis=0)
    bias_pack[ROW_BB1, 0:64] = bb1

    out_bias = np.array([[bb2b[0] - wb2b.sum()]], np.float32)

    konst = np.zeros((128, 129), np.float32)
    p = np.arange(128)
    konst[p, p % 64] = 1.0                    # Sel
    konst[np.arange(64), 64 + np.arange(64)] = 1.0   # I64
    konst[0:64, 128] = 1.0                    # ones

    import ml_dtypes
    bf16 = np.dtype(ml_dtypes.bfloat16)
    shared = dict(
        wcat=np.ascontiguousarray(wcat, np.float32),
        w1b=np.ascontiguousarray(w1b).astype(bf16),
        w2b=np.ascontiguousarray(w2b).astype(bf16),
        wb2b=np.ascontiguousarray(wb2b).astype(bf16),
        biases=bias_pack, konst=konst.astype(bf16), out_bias=out_bias,
    )

    in_maps = []
    for c in range(N_CORES):
        sl = slice(c * NC_SAMPLES, (c + 1) * NC_SAMPLES)
        xt_c = np.ascontiguousarray(st[sl].T, np.float32)          # [C, n]
        qT = np.ascontiguousarray(q[sl].T, np.float32)             # [A, n]
        qb_c = np.repeat(qT, E, axis=0).astype(bf16)               # [AE, n]
        in_maps.append(dict(xt=xt_c, qb=np.ascontiguousarray(qb_c), **shared))
    return in_maps


def kernel(agent_q_values, central_states,
           w1a, b1a, w1b, b1b, w2a, b2a, w2b, b2b,
           wb1, bb1, wb2a, bb2a, wb2b, bb2b, _trace=False, _result_box=None):
    nc = _get_nc()
    weights = (w1a, b1a, w1b, b1b, w2a, b2a, w2b, b2b,
               wb1, bb1, wb2a, bb2a, wb2b, bb2b)
    weights = tuple(np.asarray(w, np.float32) for w in weights)
    in_maps = _prep_core_inputs(
        np.asarray(agent_q_values, np.float32),
        np.asarray(central_states, np.float32), weights)

    res = run_bass_kernel_spmd(nc, in_maps, core_ids=list(range(N_CORES)),
                               trace=_trace)
    if _result_box is not None:
        _result_box.append(res)

    out = np.concatenate(
        [res.results[c]["out"].reshape(NC_SAMPLES) for c in range(N_CORES)])
    return out.reshape(B, S, 1).astype(np.float32)
